# revision 3
# baseline (speedup 1.0000x reference)
"""Bass/Trainium2 kernel for nn_CausalSelfAttention (B=2, T=2048, C=1024, 16 heads).

Sharding (8 NeuronCores): data-parallel over batch (cores 0-3 -> batch 0,
cores 4-7 -> batch 1) x tensor-parallel over heads (4 heads per core).
Each core:
  - ternarizes its w_qkv row-shard / w_out column-shard on device
    (scales replicated, computed host-side: scalar mean |w|),
  - computes qkv.T = Wt_local @ x_b.T for its 12 feature-rows-of-64,
  - causal softmax attention for its 4 (head) x 1 (batch) pairs,
  - partial output projection out_partial = attn_out_local @ Wt_out_local.T.
Host sums the 4 partials per batch (the row-split w_out reduce) and stacks.

Self-contained: only imports the concourse toolchain from /opt/trn_rl_repo.
"""

import os
import sys

if "/opt/trn_rl_repo" not in sys.path:
    sys.path.insert(0, "/opt/trn_rl_repo")

import numpy as np

import concourse.bass as bass
import concourse.tile as tile
from concourse import bacc, mybir
from concourse.bass_utils import run_bass_kernel_spmd
from concourse.masks import make_identity

F32 = mybir.dt.float32
F32R = mybir.dt.float32r
BF16 = mybir.dt.bfloat16
AF = mybir.ActivationFunctionType
ALU = mybir.AluOpType

B, T, C = 2, 2048, 1024
NH, D = 16, 64
NCORES = 8
HPC = 4          # heads per core
FQKV = 3 * HPC * D   # 768 local qkv features
JL = HPC * D         # 256 local out-proj contraction cols
NEG_FILL = 1e30

# matmul input dtype: float32r = single-pass FP22 (4x faster), float32 = exact
MM_DT = F32R if os.environ.get("KMM_DT", "f32r") == "f32r" else F32
P_BF16 = bool(int(os.environ.get("KP_BF16", "1")))
PV_DT = BF16 if P_BF16 else None  # set after MM_SD known
LAST_EXEC_TIME_NS = None
LAST_TRACE_PATH = None


MM_SD = MM_DT  # storage dtype for tiles feeding f32r matmuls
if PV_DT is None:
    PV_DT = MM_SD


def _mm(ap):
    return ap


def build():
    nc = bacc.Bacc(
        "TRN2",
        target_bir_lowering=False,
        debug=False,
        enable_asserts=False,
        num_devices=NCORES,
    )
    x_d = nc.dram_tensor("x", [T, C], F32, kind="ExternalInput").ap()
    wq_d = nc.dram_tensor("wqkv", [FQKV, C], F32, kind="ExternalInput").ap()
    wo_d = nc.dram_tensor("wout", [C, JL], F32, kind="ExternalInput").ap()
    sc_d = nc.dram_tensor("scales", [1, 2], F32, kind="ExternalInput").ap()
    y_d = nc.dram_tensor("y", [T, C], F32, kind="ExternalOutput").ap()

    NT = T // 128        # 16 token tiles
    NC_ = C // 128       # 8 contraction tiles
    NFQ = FQKV // 128    # 6 qkv feature tiles
    MB_W = T + 128       # big causal mask width

    with tile.TileContext(nc) as tc:
        const = tc.alloc_tile_pool(name="const", bufs=1)
        # persistent across phases
        qkvT_pool = tc.alloc_tile_pool(name="qkvT", bufs=NFQ)
        attnT_pool = tc.alloc_tile_pool(name="attnT", bufs=2)
        wot_pool = tc.alloc_tile_pool(name="wotT", bufs=2)
        ps_mm = tc.alloc_tile_pool(name="ps_mm", bufs=2, space="PSUM")
        ps_pt = tc.alloc_tile_pool(name="ps_pt", bufs=2, space="PSUM")
        ps_av = tc.alloc_tile_pool(name="ps_av", bufs=1, space="PSUM")

        ident = const.tile([128, 128], F32, tag="ident")
        make_identity(nc, ident)
        ident_r = ident
        if MM_SD is F32R:
            ident_r = const.tile([128, 128], F32R, tag="ident_r")
            nc.vector.tensor_copy(ident_r, ident)
        ident_pv = ident_r
        if P_BF16:
            ident_pv = const.tile([128, 128], BF16, tag="ident_pv")
            nc.vector.tensor_copy(ident_pv, ident)
        # causal mask: mb[i, c] = 0 if (T + i - c) >= 0 else +NEG_FILL.
        # For q-tile qt use cols [T - 128*qt + k]: valid (0) iff k <= 128*qt + i.
        mbig = const.tile([128, MB_W], F32, tag="mbig")
        nc.gpsimd.memset(mbig, 0.0)
        nc.gpsimd.affine_select(
            out=mbig,
            in_=mbig,
            compare_op=ALU.is_ge,
            fill=-NEG_FILL,
            base=T,
            pattern=[[-1, MB_W]],
            channel_multiplier=1,
        )
        s_sb = const.tile([128, 2], F32, tag="scales_sb")
        nc.sync.dma_start(out=s_sb, in_=sc_d.partition_broadcast(128))
        hq = const.tile([128, 1], F32, tag="hq")
        nhq = const.tile([128, 1], F32, tag="nhq")
        ho = const.tile([128, 1], F32, tag="ho")
        nho = const.tile([128, 1], F32, tag="nho")
        nc.gpsimd.tensor_scalar_mul(hq, s_sb[:, 0:1], 0.5)
        nc.gpsimd.tensor_scalar_mul(nhq, s_sb[:, 0:1], -0.5)
        nc.gpsimd.tensor_scalar_mul(ho, s_sb[:, 1:2], 0.5)
        nc.gpsimd.tensor_scalar_mul(nho, s_sb[:, 1:2], -0.5)

        qkvT = [qkvT_pool.tile([128, T], MM_SD, tag="qkvT", name=f"qkvT{i}") for i in range(NFQ)]
        attnT = [attnT_pool.tile([128, T], MM_SD, tag="attnT", name=f"attnT{i}") for i in range(2)]
        wotT = [wot_pool.tile([128, C], MM_SD, tag="wotT", name=f"wotT{i}") for i in range(2)]

        def ternarize(dst, src, thr_hi, thr_lo, scale):
            # dst = ((src > hi) - (src < lo)) * scale  ==  clip(round(src/s),-1,1)*scale
            b = tern_tmp.tile(list(src.shape), F32, tag="tern_b")
            nc.gpsimd.tensor_scalar(dst, src, thr_hi, scale, op0=ALU.is_gt, op1=ALU.mult)
            nc.gpsimd.tensor_scalar(b, src, thr_lo, scale, op0=ALU.is_lt, op1=ALU.mult)
            nc.gpsimd.tensor_tensor(out=dst, in0=dst, in1=b, op=ALU.subtract)

        # ---------------- phase W + projection ----------------
        with (
            tc.tile_pool(name="wraw", bufs=2) as wraw,
            tc.tile_pool(name="tern_tmp", bufs=2) as tern_tmp,
            tc.tile_pool(name="wt", bufs=3) as wt_pool,
            tc.tile_pool(name="wto", bufs=8) as wto_pool,
            tc.tile_pool(name="wqT", bufs=NC_) as wqT_pool,
            tc.tile_pool(name="xnat", bufs=4) as xnat_pool,
            tc.tile_pool(name="xT", bufs=NC_) as xT_pool,
        ):
            wqT = [wqT_pool.tile([128, FQKV], MM_SD, tag="wqT", name=f"wqT{i}") for i in range(NC_)]
            # w_qkv: ternarize + transpose (q rows get the 1/sqrt(D) fold = 0.125)
            for grp in range(2):  # two groups of 3 feature-tiles
                wts = []
                for k in range(3):
                    wi = 3 * grp + k
                    raw = wraw.tile([128, C], F32, tag="wraw")
                    nc.sync.dma_start(out=raw, in_=wq_d[128 * wi:128 * (wi + 1), :])
                    wt = wt_pool.tile([128, C], F32, tag="wt")
                    qscale = 0.125 if wi < 2 else 1.0
                    ternarize(wt, raw, hq, nhq, qscale)
                    wts.append(wt)
                for ci in range(NC_):
                    ps = ps_mm.tile([128, 1024], F32, tag="mm")
                    for k in range(3):
                        nc.tensor.transpose(
                            ps[:, 128 * k:128 * (k + 1)],
                            wts[k][:, 128 * ci:128 * (ci + 1)],
                            ident,
                        )
                    nc.any.tensor_copy(
                        wqT[ci][:, 384 * grp:384 * (grp + 1)], ps[:, 0:384]
                    )
            # w_out: ternarize + transpose -> wotT[ji] = [128 j, 1024 o]
            wtos = []
            for oi in range(8):
                raw = wraw.tile([128, JL], F32, tag="wraw_o")
                nc.sync.dma_start(out=raw, in_=wo_d[128 * oi:128 * (oi + 1), :])
                wto = wto_pool.tile([128, JL], F32, tag="wto")
                ternarize(wto, raw, ho, nho, 1.0)
                wtos.append(wto)
            for ji in range(2):
                for og in range(2):
                    ps = ps_mm.tile([128, 1024], F32, tag="mm")
                    for k in range(4):
                        oi = 4 * og + k
                        nc.tensor.transpose(
                            ps[:, 128 * k:128 * (k + 1)],
                            wtos[oi][:, 128 * ji:128 * (ji + 1)],
                            ident,
                        )
                    nc.any.tensor_copy(
                        wotT[ji][:, 512 * og:512 * (og + 1)], ps[:, 0:512]
                    )

            # x transpose + qkv projection, in two token-halves
            for p in range(2):
                xT = [xT_pool.tile([128, T // 2], MM_SD, tag="xT", name=f"xT{i}") for i in range(NC_)]
                for tg in range(2):
                    xns = []
                    for k in range(4):
                        ti = 8 * p + 4 * tg + k
                        xn = xnat_pool.tile([128, C], F32, tag="xnat")
                        nc.sync.dma_start(
                            out=xn, in_=x_d[128 * ti:128 * (ti + 1), :]
                        )
                        xns.append(xn)
                    for ci in range(NC_):
                        ps = ps_mm.tile([128, 1024], F32, tag="mm")
                        for k in range(4):
                            nc.tensor.transpose(
                                ps[:, 128 * k:128 * (k + 1)],
                                xns[k][:, 128 * ci:128 * (ci + 1)],
                                ident,
                            )
                        nc.any.tensor_copy(
                            xT[ci][:, 512 * tg:512 * (tg + 1)], ps[:, 0:512]
                        )
                for fi in range(NFQ):
                    ps = ps_mm.tile([128, 1024], F32, tag="mm", name="ps_qkv")
                    for ci in range(NC_):
                        st = ci == 0
                        sp = ci == NC_ - 1
                        for tj in range(2):
                            nc.tensor.matmul(
                                ps[:, 512 * tj:512 * (tj + 1)],
                                _mm(wqT[ci][:, 128 * fi:128 * (fi + 1)]),
                                _mm(xT[ci][:, 512 * tj:512 * (tj + 1)]),
                                start=st,
                                stop=sp,
                            )
                    nc.any.tensor_copy(
                        qkvT[fi][:, 1024 * p:1024 * (p + 1)], ps[:, 0:1024]
                    )

        # ---------------- attention ----------------
        with (
            tc.tile_pool(name="vh", bufs=2) as vh_pool,
            tc.tile_pool(name="sneg", bufs=3) as sneg_pool,
            tc.tile_pool(name="pp", bufs=5) as p_pool,
            tc.tile_pool(name="ptsb", bufs=3) as ptsb_pool,
            tc.tile_pool(name="tiny", bufs=8) as tiny,
        ):
            for h in range(HPC):
                fi_q, off_q = h // 2, 64 * (h % 2)
                fi_k = 2 + h // 2
                fi_v = 4 + h // 2
                qT = qkvT[fi_q][off_q:off_q + 64, :]
                kT = qkvT[fi_k][off_q:off_q + 64, :]
                vT = qkvT[fi_v][off_q:off_q + 64, :]
                # v natural layout [k-tile partitions, 64 d] per token tile
                v_h = vh_pool.tile([128, NT * 64], PV_DT, tag="vh")
                for kg in range(2):
                    ps = ps_mm.tile([128, 1024], MM_SD, tag="mm", name="ps_vtr")
                    for k in range(8):
                        kt = 8 * kg + k
                        nc.tensor.transpose(
                            ps[:, 64 * k:64 * (k + 1)],
                            vT[:, 128 * kt:128 * (kt + 1)],
                            ident_r[off_q:off_q + 64, off_q:off_q + 64],
                        )
                    nc.any.tensor_copy(v_h[:, 512 * kg:512 * (kg + 1)], ps[:, 0:512])

                for qc in range(4):
                    Ps = []
                    for j in range(4):
                        qt = 4 * qc + j
                        Lk = 128 * (qt + 1)
                        moff = T - 128 * qt
                        smask = sneg_pool.tile([128, T], F32, tag="sneg")
                        nms = []
                        nkc = (Lk + 1023) // 1024
                        for kc in range(nkc):
                            ln = min(1024, Lk - 1024 * kc)
                            ps = ps_mm.tile([128, 1024], F32, tag="mm")
                            for sub in range((ln + 511) // 512):
                                l2 = min(512, ln - 512 * sub)
                                o = 512 * sub
                                nc.tensor.matmul(
                                    ps[:, o:o + l2],
                                    _mm(qT[:, 128 * qt:128 * (qt + 1)]),
                                    _mm(kT[:, 1024 * kc + o:1024 * kc + o + l2]),
                                    start=True,
                                    stop=True,
                                )
                            nc.vector.tensor_tensor(
                                out=smask[:, 1024 * kc:1024 * kc + ln],
                                in0=ps[:, 0:ln],
                                in1=mbig[:, moff + 1024 * kc:moff + 1024 * kc + ln],
                                op=ALU.add,
                            )
                            nm_new = tiny.tile([128, 1], F32, tag="nm")
                            nc.vector.tensor_reduce(
                                nm_new,
                                smask[:, 1024 * kc:1024 * kc + ln],
                                axis=mybir.AxisListType.X,
                                op=ALU.max,
                                negate=True,
                            )
                            nms.append(nm_new)
                        if nkc == 1:
                            nm = nms[0]
                        else:
                            nm = tiny.tile([128, 1], F32, tag="nm2")
                            nc.vector.tensor_tensor(out=nm, in0=nms[0], in1=nms[1], op=ALU.min)
                        lim = 512 * (qc + 1)
                        if Lk < lim:
                            # tail: exp(-1e30 + nm) = 0 fills P beyond the causal edge
                            nc.gpsimd.memset(smask[:, Lk:lim], -NEG_FILL)
                        P_j = p_pool.tile([128, T], PV_DT, tag="P")
                        rowsum = tiny.tile([128, 1], F32, tag="rs")
                        nc.scalar.activation(
                            P_j[:, 0:lim],
                            smask[:, 0:lim],
                            AF.Exp,
                            bias=nm,
                            scale=1.0,
                            accum_out=rowsum,
                        )
                        recip = tiny.tile([128, 1], F32, tag="recip")
                        nc.vector.reciprocal(recip, rowsum)
                        nc.vector.tensor_scalar(
                            P_j[:, 0:Lk], P_j[:, 0:Lk], recip, None, op0=ALU.mult
                        )
                        Ps.append(P_j)

                    psav = ps_av.tile([64, 512], F32, tag="av")
                    nkt = 4 * (qc + 1)
                    for kt in range(nkt):
                        pspt = ps_pt.tile([128, 512], PV_DT, tag="pt")
                        for j in range(4):
                            nc.tensor.transpose(
                                pspt[:, 128 * j:128 * (j + 1)],
                                Ps[j][:, 128 * kt:128 * (kt + 1)],
                                ident_pv,
                            )
                        pt_sb = ptsb_pool.tile([128, 512], PV_DT, tag="ptsb")
                        nc.any.tensor_copy(pt_sb, pspt)
                        nc.tensor.matmul(
                            psav,
                            v_h[:, 64 * kt:64 * (kt + 1)] if P_BF16 else _mm(v_h[:, 64 * kt:64 * (kt + 1)]),
                            pt_sb if P_BF16 else _mm(pt_sb),
                            start=(kt == 0),
                            stop=(kt == nkt - 1),
                        )
                    nc.any.tensor_copy(
                        attnT[h // 2][off_q:off_q + 64, 512 * qc:512 * (qc + 1)],
                        psav,
                    )

        # ---------------- output projection (partial) ----------------
        with tc.tile_pool(name="outsb", bufs=2) as out_pool:
            for ti in range(NT):
                ps = ps_mm.tile([128, 1024], F32, tag="mm")
                for oc in range(2):
                    for ji in range(2):
                        nc.tensor.matmul(
                            ps[:, 512 * oc:512 * (oc + 1)],
                            _mm(attnT[ji][:, 128 * ti:128 * (ti + 1)]),
                            _mm(wotT[ji][:, 512 * oc:512 * (oc + 1)]),
                            start=(ji == 0),
                            stop=(ji == 1),
                        )
                out_sb = out_pool.tile([128, C], F32, tag="outsb")
                nc.any.tensor_copy(out_sb, ps)
                nc.sync.dma_start(out=y_d[128 * ti:128 * (ti + 1), :], in_=out_sb)

        # release persistent pools in reverse stack order (per memory space)
        ps_av.release()
        ps_pt.release()
        ps_mm.release()
        wot_pool.release()
        attnT_pool.release()
        qkvT_pool.release()
        const.release()

    nc.compile()
    return nc


_BUILT = None


def _get_built():
    global _BUILT
    if _BUILT is None:
        _BUILT = build()
    return _BUILT


def kernel(x, w_qkv, w_out):
    global LAST_EXEC_TIME_NS
    x = np.ascontiguousarray(np.asarray(x, dtype=np.float32))
    w_qkv = np.ascontiguousarray(np.asarray(w_qkv, dtype=np.float32))
    w_out = np.ascontiguousarray(np.asarray(w_out, dtype=np.float32))

    s_qkv = np.float32(max(np.mean(np.abs(w_qkv), dtype=np.float64), 1e-8))
    s_out = np.float32(max(np.mean(np.abs(w_out), dtype=np.float64), 1e-8))
    scales = np.array([[s_qkv, s_out]], dtype=np.float32)

    in_maps = []
    for core in range(NCORES):
        b = core // 4
        g = core % 4
        r0 = JL * g
        wq_shard = np.concatenate(
            [
                w_qkv[r0:r0 + JL],
                w_qkv[C + r0:C + r0 + JL],
                w_qkv[2 * C + r0:2 * C + r0 + JL],
            ],
            axis=0,
        )
        in_maps.append(
            {
                "x": np.ascontiguousarray(x[b]),
                "wqkv": np.ascontiguousarray(wq_shard),
                "wout": np.ascontiguousarray(w_out[:, r0:r0 + JL]),
                "scales": scales,
            }
        )

    nc = _get_built()
    trace = bool(os.environ.get("BASS_KERNEL_TRACE"))
    res = run_bass_kernel_spmd(
        nc, in_maps, core_ids=list(range(NCORES)), trace=trace
    )
    LAST_EXEC_TIME_NS = res.exec_time_ns
    global LAST_TRACE_PATH
    try:
        LAST_TRACE_PATH = res.instructions_and_trace[1] if res.instructions_and_trace else None
    except Exception:
        LAST_TRACE_PATH = None

    out = np.empty((B, T, C), dtype=np.float32)
    for b in range(B):
        parts = [res.results[4 * b + g]["y"] for g in range(4)]
        out[b] = (parts[0] + parts[1]) + (parts[2] + parts[3])
    return out

